# revision 29
# baseline (speedup 1.0000x reference)
"""Multi-head self-attention (B=4, L=2048, C=512, NH=8) on 8 Trainium2 cores.

Sharding: core c = 2*b + g owns batch b and head-group g (4 of the 8 heads,
handled as 2 head-PAIRS). Partial output projections are summed on the host.

v2 dataflow (ACT-paced design):
- A stream = (head-pair, 512-wide q chunk): 8 streams x 16 kt-groups.
- Per group: the two heads' score matmuls are K=64 row-tiles at positions
  (0,0)/(64,0) and run CONCURRENTLY in the PE array (2x over serial); both
  heads' scores land in one [128,1024] psum tile read by a single exp
  ACTIVATE (ScalarE is the pacing engine at ~1147ns/group).
- attn@V for stream s-1 (e staged in SBUF) + QKV/proj filler units
  interleave into the ACT-wait windows each group.
- Softmax denominator via a ones-column appended to V (M=65 attn@V).
- PSUM: 2x[128,1024] score tiles (4 banks) + av0/av1 + f0/f1 (4 banks).
"""

import numpy as np

import concourse.bacc as bacc
import concourse.bass as bass
import concourse.mybir as mybir
import concourse.tile as tile
from concourse import bass_utils

B, L, C, NH, HD = 4, 2048, 512, 8, 64
P = 128
NCORES = 8
GH = NH // 2        # heads per core = 4
GC = GH * HD        # group channels = 256
NCI = C // P        # c_in tiles = 4
NKT = L // P        # k tiles = 16
NCH = L // 512      # 512-wide q chunks = 4

F32 = mybir.dt.float32
BF16 = mybir.dt.bfloat16
EXP = mybir.ActivationFunctionType.Exp


def _build_body(ctx, tc, xb, wg, wp, zt):
    nc = tc.nc

    const = ctx.enter_context(tc.tile_pool(name="const", bufs=1))
    dram = ctx.enter_context(tc.tile_pool(name="dram", bufs=1, space="DRAM"))
    sps = ctx.enter_context(tc.tile_pool(name="sps", bufs=2, space="PSUM"))
    work = ctx.enter_context(tc.tile_pool(name="work", bufs=1, space="PSUM"))
    epool = ctx.enter_context(tc.tile_pool(name="epool", bufs=2))
    spool = ctx.enter_context(tc.tile_pool(name="spool", bufs=4))

    # ---- Persistent SBUF ----
    XT = [const.tile([P, 1024], BF16, tag=f"xt{i}", name=f"xt{i}") for i in range(NCI * 2)]
    QT = [const.tile([P, L], BF16, tag=f"qt{p}", name=f"qt{p}") for p in range(2)]
    KT = [const.tile([P, L], BF16, tag=f"kt{p}", name=f"kt{p}") for p in range(2)]
    VA = [const.tile([P, NKT, 2, HD + 1], BF16, tag=f"va{p}", name=f"va{p}") for p in range(2)]
    OT = [[const.tile([P, 512], BF16, tag=f"ot{p}{c}", name=f"ot{p}{c}") for c in range(NCH)]
          for p in range(2)]
    WGall = const.tile([P, NCI, 3 * GC], BF16, tag="wgall")
    WG = [WGall[:, i, :] for i in range(NCI)]
    WPk = const.tile([P, 2, C], BF16, tag="wpk")
    ZB = [const.tile([P, 512], BF16, tag=f"zb{zi}", name=f"zb{zi}")
          for zi in range(NCH * NCI)]

    # ---- input DMAs ----
    # b1 half of x goes feature-major via the (slow-to-arm) xbar transpose
    # engine; the startup-critical b0 half loads naturally and is transposed
    # on the PE below, so streams can start ~13us earlier.
    XN = [const.tile([P, 2, 512], BF16, tag=f"xn{sb}", name=f"xn{sb}") for sb in range(4)]
    IDN = const.tile([P, P], BF16, tag="idn")
    for sb in range(4):
        nc.sync.dma_start(
            out=XN[sb],
            in_=xb[sb * 256 : (sb + 1) * 256, :].rearrange("(a p) c -> p a c", p=P),
        )
    for i in range(NCI):
        nc.sync.dma_start_transpose(
            out=XT[i * 2 + 1],
            in_=xb[1024:2048, i * P : (i + 1) * P],
        )
    nc.gpsimd.dma_start(out=WGall, in_=wg.rearrange("(a p) c -> p a c", p=P))
    nc.gpsimd.dma_start(out=WPk, in_=wp.rearrange("(r p) c -> p r c", p=P))

    from concourse.masks import make_identity

    make_identity(nc, IDN)
    for p in range(2):
        nc.vector.memset(VA[p][:, :, :, HD : HD + 1], 1.0)

    # ---- PE warm-up (covers x DMA latency, primes HAM) + exp table preload ----
    wtrash = const.tile([P, P], BF16, tag="wtrash")
    nc.vector.memset(wtrash, 0.001)
    wps = work.tile([P, 512], F32, tag="f0", name="warmps")
    for w in range(2):
        nc.tensor.matmul(
            wps[0:HD, 0:P], wtrash[:, 0:HD], wtrash[:, 0:P],
            start=True, stop=True, skip_group_check=True,
        )
    wsb = const.tile([1, 8], F32, tag="wsb")
    nc.scalar.activation(wsb, wps[0:1, 0:8], EXP, scale=0.001)  # table preload

    # ---- filler units (all 128-contraction, single work slot each) ----
    def kq_unit(p, qk, c, slot):
        dst = (QT, KT)[qk]
        ps = work.tile([P, 512], F32, tag=slot, name=f"kq{p}{qk}{c}")
        for i in range(NCI):
            nc.tensor.matmul(
                ps,
                WG[i][:, qk * GC + p * P : qk * GC + (p + 1) * P],
                XT[i * 2 + c // 2][:, (c % 2) * 512 : (c % 2 + 1) * 512],
                start=(i == 0), stop=(i == NCI - 1),
                skip_group_check=True,
            )
        nc.vector.tensor_copy(out=dst[p][:, c * 512 : (c + 1) * 512], in_=ps)

    def v_unit(p, t, slot):
        ps = work.tile([P, P], F32, tag=slot, name=f"v{p}{t}")
        for i in range(NCI):
            nc.tensor.matmul(
                ps,
                XT[i * 2 + t // 8][:, (t % 8) * P : (t % 8 + 1) * P],
                WG[i][:, 2 * GC + p * P : 2 * GC + (p + 1) * P],
                start=(i == 0), stop=(i == NCI - 1),
                skip_group_check=True,
            )
        nc.vector.tensor_copy(
            out=VA[p][:, t, :, 0:HD],
            in_=ps.rearrange("p (h d) -> p h d", d=HD),
        )

    def proj_unit(c, co, slot):
        ps = work.tile([P, 512], F32, tag=slot, name=f"zp{c}{co}")
        for pr in range(2):
            nc.tensor.matmul(
                ps,
                WPk[:, pr, co * P : (co + 1) * P],
                OT[pr][c],
                start=(pr == 0), stop=(pr == 1),
                skip_group_check=True,
            )
        zi = c * NCI + co
        nc.vector.tensor_copy(out=ZB[zi], in_=ps)
        nc.sync.dma_start(
            out=zt[co * P : (co + 1) * P, c * 512 : (c + 1) * 512], in_=ZB[zi]
        )

    # ---- startup compute: PE-transpose seq 0-511 first so the startup KQ
    # units (and with them the first exp) launch as early as possible; the
    # seq 512-1023 halves follow right after.
    for i in range(NCI):
        tp = work.tile([P, 512], BF16, tag=("f0", "f1")[i % 2], name=f"tpa{i}")
        for j in range(4):
            nc.tensor.transpose(
                tp[:, j * P : (j + 1) * P],
                XN[j // 2][:, j % 2, i * P : (i + 1) * P],
                IDN,
            )
        nc.vector.tensor_copy(out=XT[i * 2][:, 0:512], in_=tp)
    kq_unit(0, 1, 0, "f0")   # KT[p0] cols 0-511 (kt 0-3)
    kq_unit(0, 0, 0, "f1")   # QT[p0] cols 0-511 (chunk 0)

    def tpb_unit(i, slot):
        # seq 512-1023 transpose halves, run as stream-0 fillers
        tp = work.tile([P, 512], BF16, tag=slot, name=f"tpb{i}")
        for j in range(4, 8):
            nc.tensor.transpose(
                tp[:, (j - 4) * P : (j - 3) * P],
                XN[j // 2][:, j % 2, i * P : (i + 1) * P],
                IDN,
            )
        nc.vector.tensor_copy(out=XT[i * 2][:, 512:1024], in_=tp)

    # ---- attention streams ----
    # stream s: (pair, chunk); per group g: av MMs for stream s-1 (+ s7 inline),
    # one filler unit, the score MM pair, the exp ACTIVATE.
    STREAMS = [(0, 0), (0, 1), (0, 2), (0, 3), (1, 0), (1, 1), (1, 2), (1, 3)]

    # filler schedule per stream: list of (fn, args) consumed one per group
    fillers = {
        0: [(tpb_unit, (0,)), (tpb_unit, (1,)), (tpb_unit, (2,)), (tpb_unit, (3,)),
            (kq_unit, (0, 1, 1)), (kq_unit, (0, 1, 2)), (kq_unit, (0, 1, 3)),
            (kq_unit, (0, 0, 1))] + [(v_unit, (0, t)) for t in range(8)],
        1: [(v_unit, (0, t)) for t in range(8, 16)] + [(kq_unit, (0, 0, 2))],
        2: [(kq_unit, (0, 0, 3)), (kq_unit, (1, 1, 0)), (kq_unit, (1, 1, 1)),
            (v_unit, (1, 0)), (v_unit, (1, 1)), (v_unit, (1, 2)), (v_unit, (1, 3))],
        3: [(kq_unit, (1, 1, 2)), (kq_unit, (1, 1, 3)), (kq_unit, (1, 0, 0))]
           + [(v_unit, (1, t)) for t in range(4, 12)],
        4: [(v_unit, (1, 12)), (v_unit, (1, 13)), (v_unit, (1, 14)),
            (v_unit, (1, 15)), (kq_unit, (1, 0, 1))],
        5: [(kq_unit, (1, 0, 2))],
        6: [(kq_unit, (1, 0, 3)),
            (proj_unit, (0, 0)), (proj_unit, (0, 1)),
            (proj_unit, (0, 2)), (proj_unit, (0, 3))],
        7: [],
    }

    e_tiles = {}      # stream idx -> e AP
    av_tiles = {}     # stream idx -> (av_A, av_B) psum APs

    def av_mms(src, g, kt):
        """attn@V matmuls for stream `src` at its kt step (M=65 incl ones)."""
        p, c = STREAMS[src]
        avA, avB = av_tiles[src]
        e_src = e_tiles[src]
        for h, av in ((0, avA), (1, avB)):
            nc.tensor.matmul(
                av,
                VA[p][:, kt, h, :],
                e_src[:, kt, h * 512 : (h + 1) * 512],
                start=(kt == 0), stop=(kt == NKT - 1),
                skip_group_check=True,
            )

    def evac_norm(src, tail=False):
        """av -> oc, fast rowsum reciprocal, DRAM-bounce broadcast, OT write."""
        p, c = STREAMS[src]
        avA, avB = av_tiles[src]
        for h, av in ((0, avA), (1, avB)):
            # at the tail both hwdge queues are free: run the two heads'
            # DMA bounce chains in parallel (scalar + gpsimd)
            q = (nc.scalar if h == 0 else nc.gpsimd) if tail else nc.gpsimd
            oc = spool.tile([HD + 1, 512], F32, tag="oc", name=f"oc{src}{h}")
            nc.vector.tensor_copy(out=oc, in_=av)
            sp = spool.tile([P, 4], F32, tag="sp", name=f"sp{src}{h}")
            q.dma_start(out=sp, in_=oc[HD : HD + 1, :])
            nc.vector.reciprocal(out=sp, in_=sp)
            rd = dram.tile([1, 512], F32, tag=f"rd{src}{h}", name=f"rd{src}{h}")
            q.dma_start(out=rd, in_=sp)
            bcast = bass.AP(tensor=rd.tensor, offset=rd.offset,
                            ap=[[0, HD]] + list(rd.ap[1:]))
            rs = spool.tile([HD, 512], F32, tag="rs", name=f"rs{src}{h}")
            q.dma_start(out=rs, in_=bcast)
            nc.vector.tensor_mul(
                out=OT[p][c][h * HD : (h + 1) * HD, :], in0=oc[0:HD, :], in1=rs
            )

    for s, (p, c) in enumerate(STREAMS):
        e_cur = epool.tile([P, NKT, 1024], BF16, tag="e", name=f"e{s}")
        e_tiles[s] = e_cur
        if s > 0:
            av_tiles[s - 1] = (
                work.tile([HD + 1, 512], F32, tag="av0", name=f"avA{s - 1}"),
                work.tile([HD + 1, 512], F32, tag="av1", name=f"avB{s - 1}"),
            )
        if s == 7:
            av_tiles[7] = (
                work.tile([HD + 1, 512], F32, tag="f0", name="avA7"),
                work.tile([HD + 1, 512], F32, tag="f1", name="avB7"),
            )
        flist = fillers[s]
        fslot = 0
        # stream 7: av(6) double-paced g0-7 so its norm starts early; av(7)
        # runs inline (lag 1 group, on the f slots); proj c1/c2 units fill
        # the freed av slots from g10.
        s7_proj = [(1, 0), (1, 1), (1, 2), (1, 3), (2, 0), (2, 1), (2, 2)]
        for g in range(NKT):
            if s == 7:
                if g < 8:
                    av_mms(6, g, 2 * g)
                    av_mms(6, g, 2 * g + 1)
                if g == 8:
                    # gpsimd queue: scalar-queue DMAs here would stall the
                    # exp stream (the scalar engine dispatches hwdge DMAs)
                    evac_norm(6, tail=False)
                if g >= 1:
                    av_mms(7, g, g - 1)
                if g >= 9:
                    pc, pco = s7_proj[g - 9]
                    proj_unit(pc, pco, ("av0", "av1")[g % 2])
            elif s > 0:
                av_mms(s - 1, g, g)
            if g < len(flist):
                fn, args = flist[g]
                fn(*args, ("f0", "f1")[fslot % 2])
                fslot += 1
            st = sps.tile([P, 1024], F32, tag="st", name=f"st{s}{g}")
            for h in range(2):
                nc.tensor.matmul(
                    st[:, h * 512 : (h + 1) * 512],
                    KT[p][h * HD : (h + 1) * HD, g * P : (g + 1) * P],
                    QT[p][h * HD : (h + 1) * HD, c * 512 : (c + 1) * 512],
                    start=True, stop=True,
                )
            nc.scalar.activation(e_cur[:, g, :], st, EXP, scale=1.0 / np.sqrt(HD))
        if s == 7:
            av_mms(7, NKT, NKT - 1)
        if 0 < s < 7:
            evac_norm(s - 1)

    # ---- tail: last av norm, remaining projections ----
    evac_norm(7, tail=True)
    # rotate over all four freed psum slots so the tail units pipeline
    tail_slots = ("av0", "av1", "f0", "f1")
    for n, (c, co) in enumerate([(2, 3), (3, 0), (3, 1), (3, 2), (3, 3)]):
        proj_unit(c, co, tail_slots[n % 4])

    # warm-up keep-alive (prevents DCE of the warm-up train)
    wdr = dram.tile([1, 8], F32, tag="wdr", name="wdr")
    nc.sync.dma_start(out=wdr, in_=wsb)


_CACHE = {}


def _get_nc():
    if "nc" in _CACHE:
        return _CACHE["nc"]
    nc = bacc.Bacc("TRN2", target_bir_lowering=False, debug=False)
    xb = nc.dram_tensor("xb", (L, C), BF16, kind="ExternalInput").ap()
    wg = nc.dram_tensor("wg", (C, 3 * GC), BF16, kind="ExternalInput").ap()
    wp = nc.dram_tensor("wp", (GC, C), BF16, kind="ExternalInput").ap()
    zt = nc.dram_tensor("zt", (C, L), BF16, kind="ExternalOutput").ap()
    from contextlib import ExitStack

    with tile.TileContext(nc) as tc, ExitStack() as ctx:
        _build_body(ctx, tc, xb, wg, wp, zt)
    nc.compile()
    _CACHE["nc"] = nc
    return nc


def make_in_maps(x, w_qkv, w_proj):
    """Slice full inputs into the 8 per-core input maps (pre-cast to bf16)."""
    import ml_dtypes

    bf = ml_dtypes.bfloat16
    x = np.asarray(x, dtype=np.float32).astype(bf)
    w_qkv = np.asarray(w_qkv, dtype=np.float32).astype(bf)
    w_proj = np.asarray(w_proj, dtype=np.float32).astype(bf)
    in_maps = []
    for c in range(NCORES):
        b, g = divmod(c, 2)
        cols = slice(g * GC, (g + 1) * GC)
        wg_c = np.concatenate(
            [w_qkv[:, cols], w_qkv[:, C + g * GC : C + (g + 1) * GC],
             w_qkv[:, 2 * C + g * GC : 2 * C + (g + 1) * GC]],
            axis=1,
        )
        in_maps.append(
            {
                "xb": np.ascontiguousarray(x[b]),
                "wg": np.ascontiguousarray(wg_c),
                "wp": np.ascontiguousarray(w_proj[cols, :]),
            }
        )
    return in_maps


def gather_output(results, b_proj):
    out = np.empty((B, L, C), dtype=np.float32)
    for b in range(B):
        z = (results[2 * b]["zt"].astype(np.float32)
             + results[2 * b + 1]["zt"].astype(np.float32))  # [C, L]
        out[b] = z.T + b_proj[None, :]
    return out


def kernel(x, w_qkv, b_qkv, w_proj, b_proj, _trace=False):
    assert np.abs(np.asarray(b_qkv)).max() == 0.0, "kernel assumes b_qkv == 0"
    nc = _get_nc()
    in_maps = make_in_maps(x, w_qkv, w_proj)
    res = bass_utils.run_bass_kernel_spmd(
        nc, in_maps, core_ids=list(range(NCORES)), trace=_trace
    )
    out = gather_output(res.results, np.asarray(b_proj, dtype=np.float32))
    if _trace:
        return out, res
    return out


# revision 30
# speedup vs baseline: 1.1358x; 1.1358x over previous
"""Multi-head self-attention (B=4, L=2048, C=512, NH=8) on 8 Trainium2 cores.

Sharding: core c = 2*b + g owns batch b and head-group g (4 of the 8 heads,
handled as 2 head-PAIRS). Partial output projections are summed on the host.

v2 dataflow (ACT-paced design):
- A stream = (head-pair, 512-wide q chunk): 8 streams x 16 kt-groups.
- Per group: the two heads' score matmuls are K=64 row-tiles at positions
  (0,0)/(64,0) and run CONCURRENTLY in the PE array (2x over serial); both
  heads' scores land in one [128,1024] psum tile read by a single exp
  ACTIVATE (ScalarE is the pacing engine at ~1147ns/group).
- attn@V for stream s-1 (e staged in SBUF) + QKV/proj filler units
  interleave into the ACT-wait windows each group.
- Softmax denominator via a ones-column appended to V (M=65 attn@V).
- PSUM: 2x[128,1024] score tiles (4 banks) + av0/av1 + f0/f1 (4 banks).
"""

import numpy as np

import concourse.bacc as bacc
import concourse.bass as bass
import concourse.mybir as mybir
import concourse.tile as tile
from concourse import bass_utils

B, L, C, NH, HD = 4, 2048, 512, 8, 64
P = 128
NCORES = 8
GH = NH // 2        # heads per core = 4
GC = GH * HD        # group channels = 256
NCI = C // P        # c_in tiles = 4
NKT = L // P        # k tiles = 16
NCH = L // 512      # 512-wide q chunks = 4

F32 = mybir.dt.float32
BF16 = mybir.dt.bfloat16
EXP = mybir.ActivationFunctionType.Exp


def _build_body(ctx, tc, xb, wg, wp, zt):
    nc = tc.nc

    const = ctx.enter_context(tc.tile_pool(name="const", bufs=1))
    dram = ctx.enter_context(tc.tile_pool(name="dram", bufs=1, space="DRAM"))
    sps = ctx.enter_context(tc.tile_pool(name="sps", bufs=2, space="PSUM"))
    work = ctx.enter_context(tc.tile_pool(name="work", bufs=1, space="PSUM"))
    epool = ctx.enter_context(tc.tile_pool(name="epool", bufs=2))
    spool = ctx.enter_context(tc.tile_pool(name="spool", bufs=4))

    # ---- Persistent SBUF ----
    XT = [const.tile([P, 1024], BF16, tag=f"xt{i}", name=f"xt{i}") for i in range(NCI * 2)]
    QT = [const.tile([P, L], BF16, tag=f"qt{p}", name=f"qt{p}") for p in range(2)]
    KT = [const.tile([P, L], BF16, tag=f"kt{p}", name=f"kt{p}") for p in range(2)]
    VA = [const.tile([P, NKT, 2, HD + 1], BF16, tag=f"va{p}", name=f"va{p}") for p in range(2)]
    OT = [[const.tile([P, 512], BF16, tag=f"ot{p}{c}", name=f"ot{p}{c}") for c in range(NCH)]
          for p in range(2)]
    WGall = const.tile([P, NCI, 3 * GC], BF16, tag="wgall")
    WG = [WGall[:, i, :] for i in range(NCI)]
    WPk = const.tile([P, 2, C], BF16, tag="wpk")
    ZB = [const.tile([P, 512], BF16, tag=f"zb{zi}", name=f"zb{zi}")
          for zi in range(NCH * NCI)]

    # ---- input DMAs ----
    # b1 half of x goes feature-major via the (slow-to-arm) xbar transpose
    # engine; the startup-critical b0 half loads naturally and is transposed
    # on the PE below, so streams can start ~13us earlier.
    XN = [const.tile([P, 2, 512], BF16, tag=f"xn{sb}", name=f"xn{sb}") for sb in range(4)]
    IDN = const.tile([P, P], BF16, tag="idn")
    for sb in range(4):
        nc.sync.dma_start(
            out=XN[sb],
            in_=xb[sb * 256 : (sb + 1) * 256, :].rearrange("(a p) c -> p a c", p=P),
        )
    for i in range(NCI):
        nc.sync.dma_start_transpose(
            out=XT[i * 2 + 1],
            in_=xb[1024:2048, i * P : (i + 1) * P],
        )
    nc.gpsimd.dma_start(out=WGall, in_=wg.rearrange("(a p) c -> p a c", p=P))
    nc.gpsimd.dma_start(out=WPk, in_=wp.rearrange("(r p) c -> p r c", p=P))

    from concourse.masks import make_identity

    make_identity(nc, IDN)
    for p in range(2):
        nc.vector.memset(VA[p][:, :, :, HD : HD + 1], 1.0)

    # ---- PE warm-up (covers x DMA latency, primes HAM) + exp table preload ----
    wtrash = const.tile([P, P], BF16, tag="wtrash")
    nc.vector.memset(wtrash, 0.001)
    # ~36 x 107ns cold MMs ≈ 3.9us of sustained PE activity: flips the HAM
    # clock gate to 8/8 before the transpose->KQ->score startup chain, which
    # then runs at 2.4GHz instead of 1.2. Overlaps the x-DMA wait, so free.
    wps = work.tile([P, 512], F32, tag="f0", name="warmps")
    for w in range(36):
        nc.tensor.matmul(
            wps[0:HD, 0:P], wtrash[:, 0:HD], wtrash[:, 0:P],
            start=True, stop=True, skip_group_check=True,
        )
    wsb = const.tile([1, 8], F32, tag="wsb")
    nc.scalar.activation(wsb, wps[0:1, 0:8], EXP, scale=0.001)  # table preload

    # ---- filler units (all 128-contraction, single work slot each) ----
    def kq_unit(p, qk, c, slot):
        dst = (QT, KT)[qk]
        ps = work.tile([P, 512], F32, tag=slot, name=f"kq{p}{qk}{c}")
        for i in range(NCI):
            nc.tensor.matmul(
                ps,
                WG[i][:, qk * GC + p * P : qk * GC + (p + 1) * P],
                XT[i * 2 + c // 2][:, (c % 2) * 512 : (c % 2 + 1) * 512],
                start=(i == 0), stop=(i == NCI - 1),
                skip_group_check=True,
            )
        nc.vector.tensor_copy(out=dst[p][:, c * 512 : (c + 1) * 512], in_=ps)

    def v_unit(p, t, slot):
        ps = work.tile([P, P], F32, tag=slot, name=f"v{p}{t}")
        for i in range(NCI):
            nc.tensor.matmul(
                ps,
                XT[i * 2 + t // 8][:, (t % 8) * P : (t % 8 + 1) * P],
                WG[i][:, 2 * GC + p * P : 2 * GC + (p + 1) * P],
                start=(i == 0), stop=(i == NCI - 1),
                skip_group_check=True,
            )
        nc.vector.tensor_copy(
            out=VA[p][:, t, :, 0:HD],
            in_=ps.rearrange("p (h d) -> p h d", d=HD),
        )

    def proj_unit(c, co, slot):
        ps = work.tile([P, 512], F32, tag=slot, name=f"zp{c}{co}")
        for pr in range(2):
            nc.tensor.matmul(
                ps,
                WPk[:, pr, co * P : (co + 1) * P],
                OT[pr][c],
                start=(pr == 0), stop=(pr == 1),
                skip_group_check=True,
            )
        zi = c * NCI + co
        nc.vector.tensor_copy(out=ZB[zi], in_=ps)
        nc.sync.dma_start(
            out=zt[co * P : (co + 1) * P, c * 512 : (c + 1) * 512], in_=ZB[zi]
        )

    # ---- startup compute: PE-transpose seq 0-511 first so the startup KQ
    # units (and with them the first exp) launch as early as possible; the
    # seq 512-1023 halves follow right after.
    for i in range(NCI):
        tp = work.tile([P, 512], BF16, tag=("f0", "f1")[i % 2], name=f"tpa{i}")
        for j in range(4):
            nc.tensor.transpose(
                tp[:, j * P : (j + 1) * P],
                XN[j // 2][:, j % 2, i * P : (i + 1) * P],
                IDN,
            )
        nc.vector.tensor_copy(out=XT[i * 2][:, 0:512], in_=tp)
    kq_unit(0, 1, 0, "f0")   # KT[p0] cols 0-511 (kt 0-3)
    kq_unit(0, 0, 0, "f1")   # QT[p0] cols 0-511 (chunk 0)

    def tpb_unit(i, slot):
        # seq 512-1023 transpose halves, run as stream-0 fillers
        tp = work.tile([P, 512], BF16, tag=slot, name=f"tpb{i}")
        for j in range(4, 8):
            nc.tensor.transpose(
                tp[:, (j - 4) * P : (j - 3) * P],
                XN[j // 2][:, j % 2, i * P : (i + 1) * P],
                IDN,
            )
        nc.vector.tensor_copy(out=XT[i * 2][:, 512:1024], in_=tp)

    # ---- attention streams ----
    # stream s: (pair, chunk); per group g: av MMs for stream s-1 (+ s7 inline),
    # one filler unit, the score MM pair, the exp ACTIVATE.
    STREAMS = [(0, 0), (0, 1), (0, 2), (0, 3), (1, 0), (1, 1), (1, 2), (1, 3)]

    # filler schedule per stream: list of (fn, args) consumed one per group
    fillers = {
        0: [(tpb_unit, (0,)), (tpb_unit, (1,)), (tpb_unit, (2,)), (tpb_unit, (3,)),
            (kq_unit, (0, 1, 1)), (kq_unit, (0, 1, 2)), (kq_unit, (0, 1, 3)),
            (kq_unit, (0, 0, 1))] + [(v_unit, (0, t)) for t in range(8)],
        1: [(v_unit, (0, t)) for t in range(8, 16)] + [(kq_unit, (0, 0, 2))],
        2: [(kq_unit, (0, 0, 3)), (kq_unit, (1, 1, 0)), (kq_unit, (1, 1, 1)),
            (v_unit, (1, 0)), (v_unit, (1, 1)), (v_unit, (1, 2)), (v_unit, (1, 3))],
        3: [(kq_unit, (1, 1, 2)), (kq_unit, (1, 1, 3)), (kq_unit, (1, 0, 0))]
           + [(v_unit, (1, t)) for t in range(4, 12)],
        4: [(v_unit, (1, 12)), (v_unit, (1, 13)), (v_unit, (1, 14)),
            (v_unit, (1, 15)), (kq_unit, (1, 0, 1))],
        5: [(kq_unit, (1, 0, 2))],
        6: [(kq_unit, (1, 0, 3)),
            (proj_unit, (0, 0)), (proj_unit, (0, 1)),
            (proj_unit, (0, 2)), (proj_unit, (0, 3))],
        7: [],
    }

    e_tiles = {}      # stream idx -> e AP
    av_tiles = {}     # stream idx -> (av_A, av_B) psum APs

    def av_mms(src, g, kt):
        """attn@V matmuls for stream `src` at its kt step (M=65 incl ones)."""
        p, c = STREAMS[src]
        avA, avB = av_tiles[src]
        e_src = e_tiles[src]
        for h, av in ((0, avA), (1, avB)):
            nc.tensor.matmul(
                av,
                VA[p][:, kt, h, :],
                e_src[:, kt, h * 512 : (h + 1) * 512],
                start=(kt == 0), stop=(kt == NKT - 1),
                skip_group_check=True,
            )

    def evac_norm(src, tail=False):
        """av -> oc, fast rowsum reciprocal, DRAM-bounce broadcast, OT write."""
        p, c = STREAMS[src]
        avA, avB = av_tiles[src]
        for h, av in ((0, avA), (1, avB)):
            # at the tail both hwdge queues are free: run the two heads'
            # DMA bounce chains in parallel (scalar + gpsimd)
            q = (nc.scalar if h == 0 else nc.gpsimd) if tail else nc.gpsimd
            oc = spool.tile([HD + 1, 512], F32, tag="oc", name=f"oc{src}{h}")
            nc.vector.tensor_copy(out=oc, in_=av)
            sp = spool.tile([P, 4], F32, tag="sp", name=f"sp{src}{h}")
            q.dma_start(out=sp, in_=oc[HD : HD + 1, :])
            nc.vector.reciprocal(out=sp, in_=sp)
            rd = dram.tile([1, 512], F32, tag=f"rd{src}{h}", name=f"rd{src}{h}")
            q.dma_start(out=rd, in_=sp)
            bcast = bass.AP(tensor=rd.tensor, offset=rd.offset,
                            ap=[[0, HD]] + list(rd.ap[1:]))
            rs = spool.tile([HD, 512], F32, tag="rs", name=f"rs{src}{h}")
            q.dma_start(out=rs, in_=bcast)
            nc.vector.tensor_mul(
                out=OT[p][c][h * HD : (h + 1) * HD, :], in0=oc[0:HD, :], in1=rs
            )

    for s, (p, c) in enumerate(STREAMS):
        e_cur = epool.tile([P, NKT, 1024], BF16, tag="e", name=f"e{s}")
        e_tiles[s] = e_cur
        if s > 0:
            av_tiles[s - 1] = (
                work.tile([HD + 1, 512], F32, tag="av0", name=f"avA{s - 1}"),
                work.tile([HD + 1, 512], F32, tag="av1", name=f"avB{s - 1}"),
            )
        if s == 7:
            av_tiles[7] = (
                work.tile([HD + 1, 512], F32, tag="f0", name="avA7"),
                work.tile([HD + 1, 512], F32, tag="f1", name="avB7"),
            )
        flist = fillers[s]
        fslot = 0
        # stream 7: av(6) double-paced g0-7 so its norm starts early; av(7)
        # runs inline (lag 1 group, on the f slots); proj c1/c2 units fill
        # the freed av slots from g10.
        s7_proj = [(1, 0), (1, 1), (1, 2), (1, 3), (2, 0), (2, 1), (2, 2)]
        for g in range(NKT):
            if s == 7:
                if g < 8:
                    av_mms(6, g, 2 * g)
                    av_mms(6, g, 2 * g + 1)
                if g == 8:
                    # gpsimd queue: scalar-queue DMAs here would stall the
                    # exp stream (the scalar engine dispatches hwdge DMAs)
                    evac_norm(6, tail=False)
                if g >= 1:
                    av_mms(7, g, g - 1)
                if g >= 9:
                    pc, pco = s7_proj[g - 9]
                    proj_unit(pc, pco, ("av0", "av1")[g % 2])
            elif s > 0:
                av_mms(s - 1, g, g)
            if g < len(flist):
                fn, args = flist[g]
                fn(*args, ("f0", "f1")[fslot % 2])
                fslot += 1
            st = sps.tile([P, 1024], F32, tag="st", name=f"st{s}{g}")
            for h in range(2):
                nc.tensor.matmul(
                    st[:, h * 512 : (h + 1) * 512],
                    KT[p][h * HD : (h + 1) * HD, g * P : (g + 1) * P],
                    QT[p][h * HD : (h + 1) * HD, c * 512 : (c + 1) * 512],
                    start=True, stop=True,
                )
            nc.scalar.activation(e_cur[:, g, :], st, EXP, scale=1.0 / np.sqrt(HD))
        if s == 7:
            av_mms(7, NKT, NKT - 1)
        if 0 < s < 7:
            evac_norm(s - 1)

    # ---- tail: last av norm, remaining projections ----
    evac_norm(7, tail=True)
    # rotate over all four freed psum slots so the tail units pipeline
    tail_slots = ("av0", "av1", "f0", "f1")
    for n, (c, co) in enumerate([(2, 3), (3, 0), (3, 1), (3, 2), (3, 3)]):
        proj_unit(c, co, tail_slots[n % 4])

    # warm-up keep-alive (prevents DCE of the warm-up train)
    wdr = dram.tile([1, 8], F32, tag="wdr", name="wdr")
    nc.sync.dma_start(out=wdr, in_=wsb)


_CACHE = {}


def _get_nc():
    if "nc" in _CACHE:
        return _CACHE["nc"]
    nc = bacc.Bacc("TRN2", target_bir_lowering=False, debug=False)
    xb = nc.dram_tensor("xb", (L, C), BF16, kind="ExternalInput").ap()
    wg = nc.dram_tensor("wg", (C, 3 * GC), BF16, kind="ExternalInput").ap()
    wp = nc.dram_tensor("wp", (GC, C), BF16, kind="ExternalInput").ap()
    zt = nc.dram_tensor("zt", (C, L), BF16, kind="ExternalOutput").ap()
    from contextlib import ExitStack

    with tile.TileContext(nc) as tc, ExitStack() as ctx:
        _build_body(ctx, tc, xb, wg, wp, zt)
    nc.compile()
    _CACHE["nc"] = nc
    return nc


def make_in_maps(x, w_qkv, w_proj):
    """Slice full inputs into the 8 per-core input maps (pre-cast to bf16)."""
    import ml_dtypes

    bf = ml_dtypes.bfloat16
    x = np.asarray(x, dtype=np.float32).astype(bf)
    w_qkv = np.asarray(w_qkv, dtype=np.float32).astype(bf)
    w_proj = np.asarray(w_proj, dtype=np.float32).astype(bf)
    in_maps = []
    for c in range(NCORES):
        b, g = divmod(c, 2)
        cols = slice(g * GC, (g + 1) * GC)
        wg_c = np.concatenate(
            [w_qkv[:, cols], w_qkv[:, C + g * GC : C + (g + 1) * GC],
             w_qkv[:, 2 * C + g * GC : 2 * C + (g + 1) * GC]],
            axis=1,
        )
        in_maps.append(
            {
                "xb": np.ascontiguousarray(x[b]),
                "wg": np.ascontiguousarray(wg_c),
                "wp": np.ascontiguousarray(w_proj[cols, :]),
            }
        )
    return in_maps


def gather_output(results, b_proj):
    out = np.empty((B, L, C), dtype=np.float32)
    for b in range(B):
        z = (results[2 * b]["zt"].astype(np.float32)
             + results[2 * b + 1]["zt"].astype(np.float32))  # [C, L]
        out[b] = z.T + b_proj[None, :]
    return out


def kernel(x, w_qkv, b_qkv, w_proj, b_proj, _trace=False):
    assert np.abs(np.asarray(b_qkv)).max() == 0.0, "kernel assumes b_qkv == 0"
    nc = _get_nc()
    in_maps = make_in_maps(x, w_qkv, w_proj)
    res = bass_utils.run_bass_kernel_spmd(
        nc, in_maps, core_ids=list(range(NCORES)), trace=_trace
    )
    out = gather_output(res.results, np.asarray(b_proj, dtype=np.float32))
    if _trace:
        return out, res
    return out


# revision 32
# speedup vs baseline: 1.1636x; 1.0245x over previous
"""Multi-head self-attention (B=4, L=2048, C=512, NH=8) on 8 Trainium2 cores.

Sharding: core c = 2*b + g owns batch b and head-group g (4 of the 8 heads,
handled as 2 head-PAIRS). Partial output projections are summed on the host.

v2 dataflow (ACT-paced design):
- A stream = (head-pair, 512-wide q chunk): 8 streams x 16 kt-groups.
- Per group: the two heads' score matmuls are K=64 row-tiles at positions
  (0,0)/(64,0) and run CONCURRENTLY in the PE array (2x over serial); both
  heads' scores land in one [128,1024] psum tile read by a single exp
  ACTIVATE (ScalarE is the pacing engine at ~1147ns/group).
- attn@V for stream s-1 (e staged in SBUF) + QKV/proj filler units
  interleave into the ACT-wait windows each group.
- Softmax denominator via a ones-column appended to V (M=65 attn@V).
- PSUM: 2x[128,1024] score tiles (4 banks) + av0/av1 + f0/f1 (4 banks).
"""

import numpy as np

import concourse.bacc as bacc
import concourse.bass as bass
import concourse.mybir as mybir
import concourse.tile as tile
from concourse import bass_utils

B, L, C, NH, HD = 4, 2048, 512, 8, 64
P = 128
NCORES = 8
GH = NH // 2        # heads per core = 4
GC = GH * HD        # group channels = 256
NCI = C // P        # c_in tiles = 4
NKT = L // P        # k tiles = 16
NCH = L // 512      # 512-wide q chunks = 4

F32 = mybir.dt.float32
BF16 = mybir.dt.bfloat16
EXP = mybir.ActivationFunctionType.Exp


def _build_body(ctx, tc, xb, wg, wp, zt):
    nc = tc.nc

    const = ctx.enter_context(tc.tile_pool(name="const", bufs=1))
    dram = ctx.enter_context(tc.tile_pool(name="dram", bufs=1, space="DRAM"))
    sps = ctx.enter_context(tc.tile_pool(name="sps", bufs=2, space="PSUM"))
    work = ctx.enter_context(tc.tile_pool(name="work", bufs=1, space="PSUM"))
    epool = ctx.enter_context(tc.tile_pool(name="epool", bufs=2))
    spool = ctx.enter_context(tc.tile_pool(name="spool", bufs=4))

    # ---- Persistent SBUF ----
    XT = [const.tile([P, 1024], BF16, tag=f"xt{i}", name=f"xt{i}") for i in range(NCI * 2)]
    QT = [const.tile([P, L], BF16, tag=f"qt{p}", name=f"qt{p}") for p in range(2)]
    KT = [const.tile([P, L], BF16, tag=f"kt{p}", name=f"kt{p}") for p in range(2)]
    VA = [const.tile([P, NKT, 2, HD + 1], BF16, tag=f"va{p}", name=f"va{p}") for p in range(2)]
    OT = [[const.tile([P, 512], BF16, tag=f"ot{p}{c}", name=f"ot{p}{c}") for c in range(NCH)]
          for p in range(2)]
    WGall = const.tile([P, NCI, 3 * GC], BF16, tag="wgall")
    WG = [WGall[:, i, :] for i in range(NCI)]
    WPk = const.tile([P, 2, C], BF16, tag="wpk")
    ZB = [const.tile([P, 512], BF16, tag=f"zb{zi}", name=f"zb{zi}")
          for zi in range(NCH * NCI)]

    # ---- input DMAs ----
    # b1 half of x goes feature-major via the (slow-to-arm) xbar transpose
    # engine; the startup-critical b0 half loads naturally and is transposed
    # on the PE below, so streams can start ~13us earlier.
    XN = [const.tile([P, 2, 512], BF16, tag=f"xn{sb}", name=f"xn{sb}") for sb in range(4)]
    IDN = const.tile([P, P], BF16, tag="idn")
    for sb in range(4):
        nc.sync.dma_start(
            out=XN[sb],
            in_=xb[sb * 256 : (sb + 1) * 256, :].rearrange("(a p) c -> p a c", p=P),
        )
    for i in range(NCI):
        nc.sync.dma_start_transpose(
            out=XT[i * 2 + 1],
            in_=xb[1024:2048, i * P : (i + 1) * P],
        )
    nc.gpsimd.dma_start(out=WGall, in_=wg.rearrange("(a p) c -> p a c", p=P))
    nc.gpsimd.dma_start(out=WPk, in_=wp.rearrange("(r p) c -> p r c", p=P))

    from concourse.masks import make_identity

    make_identity(nc, IDN)
    for p in range(2):
        nc.vector.memset(VA[p][:, :, :, HD : HD + 1], 1.0)

    # ---- PE warm-up (covers x DMA latency, primes HAM) + exp table preload ----
    wtrash = const.tile([P, P], BF16, tag="wtrash")
    nc.vector.memset(wtrash, 0.001)
    # warmup lives on av0 (idle until stream 1) so the f0/f1 transpose units
    # aren't blocked behind the exp-table-preload ACTIVATE's read of wps
    wps = work.tile([P, 512], F32, tag="av0", name="warmps")
    for w in range(6):
        nc.tensor.matmul(
            wps[0:HD, 0:P], wtrash[:, 0:HD], wtrash[:, 0:P],
            start=True, stop=True, skip_group_check=True,
        )
    wsb = const.tile([1, 8], F32, tag="wsb")
    nc.scalar.activation(wsb, wps[0:1, 0:8], EXP, scale=0.001)  # table preload

    # ---- filler units (all 128-contraction, single work slot each) ----
    def kq_unit(p, qk, c, slot):
        dst = (QT, KT)[qk]
        ps = work.tile([P, 512], F32, tag=slot, name=f"kq{p}{qk}{c}")
        for i in range(NCI):
            nc.tensor.matmul(
                ps,
                WG[i][:, qk * GC + p * P : qk * GC + (p + 1) * P],
                XT[i * 2 + c // 2][:, (c % 2) * 512 : (c % 2 + 1) * 512],
                start=(i == 0), stop=(i == NCI - 1),
                skip_group_check=True,
            )
        nc.vector.tensor_copy(out=dst[p][:, c * 512 : (c + 1) * 512], in_=ps)

    def v_unit(p, t, slot):
        ps = work.tile([P, P], F32, tag=slot, name=f"v{p}{t}")
        for i in range(NCI):
            nc.tensor.matmul(
                ps,
                XT[i * 2 + t // 8][:, (t % 8) * P : (t % 8 + 1) * P],
                WG[i][:, 2 * GC + p * P : 2 * GC + (p + 1) * P],
                start=(i == 0), stop=(i == NCI - 1),
                skip_group_check=True,
            )
        nc.vector.tensor_copy(
            out=VA[p][:, t, :, 0:HD],
            in_=ps.rearrange("p (h d) -> p h d", d=HD),
        )

    def proj_unit(c, co, slot):
        ps = work.tile([P, 512], F32, tag=slot, name=f"zp{c}{co}")
        for pr in range(2):
            nc.tensor.matmul(
                ps,
                WPk[:, pr, co * P : (co + 1) * P],
                OT[pr][c],
                start=(pr == 0), stop=(pr == 1),
                skip_group_check=True,
            )
        zi = c * NCI + co
        nc.vector.tensor_copy(out=ZB[zi], in_=ps)
        nc.sync.dma_start(
            out=zt[co * P : (co + 1) * P, c * 512 : (c + 1) * 512], in_=ZB[zi]
        )

    # ---- startup compute: PE-transpose seq 0-511 first so the startup KQ
    # units (and with them the first exp) launch as early as possible; the
    # seq 512-1023 halves follow right after.
    for i in range(NCI):
        tp = work.tile([P, 512], BF16, tag=("f0", "f1")[i % 2], name=f"tpa{i}")
        for j in range(4):
            nc.tensor.transpose(
                tp[:, j * P : (j + 1) * P],
                XN[j // 2][:, j % 2, i * P : (i + 1) * P],
                IDN,
            )
        nc.vector.tensor_copy(out=XT[i * 2][:, 0:512], in_=tp)
    kq_unit(0, 1, 0, "f0")   # KT[p0] cols 0-511 (kt 0-3)
    kq_unit(0, 0, 0, "f1")   # QT[p0] cols 0-511 (chunk 0)

    def tpb_unit(i, slot):
        # seq 512-1023 transpose halves, run as stream-0 fillers
        tp = work.tile([P, 512], BF16, tag=slot, name=f"tpb{i}")
        for j in range(4, 8):
            nc.tensor.transpose(
                tp[:, (j - 4) * P : (j - 3) * P],
                XN[j // 2][:, j % 2, i * P : (i + 1) * P],
                IDN,
            )
        nc.vector.tensor_copy(out=XT[i * 2][:, 512:1024], in_=tp)

    # ---- attention streams ----
    # stream s: (pair, chunk); per group g: av MMs for stream s-1 (+ s7 inline),
    # one filler unit, the score MM pair, the exp ACTIVATE.
    STREAMS = [(0, 0), (0, 1), (0, 2), (0, 3), (1, 0), (1, 1), (1, 2), (1, 3)]

    # filler schedule per stream: list of (fn, args) consumed one per group
    fillers = {
        0: [(tpb_unit, (0,)), (tpb_unit, (1,)), (tpb_unit, (2,)), (tpb_unit, (3,)),
            (kq_unit, (0, 1, 1)), (kq_unit, (0, 1, 2)), (kq_unit, (0, 1, 3)),
            (kq_unit, (0, 0, 1))] + [(v_unit, (0, t)) for t in range(8)],
        1: [(v_unit, (0, t)) for t in range(8, 16)] + [(kq_unit, (0, 0, 2))],
        2: [(kq_unit, (0, 0, 3)), (kq_unit, (1, 1, 0)), (kq_unit, (1, 1, 1)),
            (v_unit, (1, 0)), (v_unit, (1, 1)), (v_unit, (1, 2)), (v_unit, (1, 3))],
        3: [(kq_unit, (1, 1, 2)), (kq_unit, (1, 1, 3)), (kq_unit, (1, 0, 0))]
           + [(v_unit, (1, t)) for t in range(4, 12)],
        4: [(v_unit, (1, 12)), (v_unit, (1, 13)), (v_unit, (1, 14)),
            (v_unit, (1, 15)), (kq_unit, (1, 0, 1))],
        5: [(kq_unit, (1, 0, 2))],
        6: [(kq_unit, (1, 0, 3)),
            (proj_unit, (0, 0)), (proj_unit, (0, 1)),
            (proj_unit, (0, 2)), (proj_unit, (0, 3))],
        7: [],
    }

    e_tiles = {}      # stream idx -> e AP
    av_tiles = {}     # stream idx -> (av_A, av_B) psum APs

    def av_mms(src, g, kt):
        """attn@V matmuls for stream `src` at its kt step (M=65 incl ones)."""
        p, c = STREAMS[src]
        avA, avB = av_tiles[src]
        e_src = e_tiles[src]
        for h, av in ((0, avA), (1, avB)):
            nc.tensor.matmul(
                av,
                VA[p][:, kt, h, :],
                e_src[:, kt, h * 512 : (h + 1) * 512],
                start=(kt == 0), stop=(kt == NKT - 1),
                skip_group_check=True,
            )

    def evac_norm(src, tail=False):
        """av -> oc, fast rowsum reciprocal, DRAM-bounce broadcast, OT write."""
        p, c = STREAMS[src]
        avA, avB = av_tiles[src]
        for h, av in ((0, avA), (1, avB)):
            # at the tail both hwdge queues are free: run the two heads'
            # DMA bounce chains in parallel (scalar + gpsimd)
            q = (nc.scalar if h == 0 else nc.gpsimd) if tail else nc.gpsimd
            oc = spool.tile([HD + 1, 512], F32, tag="oc", name=f"oc{src}{h}")
            nc.vector.tensor_copy(out=oc, in_=av)
            sp = spool.tile([P, 4], F32, tag="sp", name=f"sp{src}{h}")
            q.dma_start(out=sp, in_=oc[HD : HD + 1, :])
            nc.vector.reciprocal(out=sp, in_=sp)
            rd = dram.tile([1, 512], F32, tag=f"rd{src}{h}", name=f"rd{src}{h}")
            q.dma_start(out=rd, in_=sp)
            bcast = bass.AP(tensor=rd.tensor, offset=rd.offset,
                            ap=[[0, HD]] + list(rd.ap[1:]))
            rs = spool.tile([HD, 512], F32, tag="rs", name=f"rs{src}{h}")
            q.dma_start(out=rs, in_=bcast)
            nc.vector.tensor_mul(
                out=OT[p][c][h * HD : (h + 1) * HD, :], in0=oc[0:HD, :], in1=rs
            )

    for s, (p, c) in enumerate(STREAMS):
        e_cur = epool.tile([P, NKT, 1024], BF16, tag="e", name=f"e{s}")
        e_tiles[s] = e_cur
        if s > 0:
            av_tiles[s - 1] = (
                work.tile([HD + 1, 512], F32, tag="av0", name=f"avA{s - 1}"),
                work.tile([HD + 1, 512], F32, tag="av1", name=f"avB{s - 1}"),
            )
        if s == 7:
            av_tiles[7] = (
                work.tile([HD + 1, 512], F32, tag="f0", name="avA7"),
                work.tile([HD + 1, 512], F32, tag="f1", name="avB7"),
            )
        flist = fillers[s]
        fslot = 0
        # stream 7: av(6) double-paced g0-7 so its norm starts early; av(7)
        # runs inline (lag 1 group, on the f slots); proj c1/c2 units fill
        # the freed av slots from g10.
        s7_proj = [(1, 0), (1, 1), (1, 2), (1, 3), (2, 0), (2, 1), (2, 2)]
        for g in range(NKT):
            if s == 7:
                if g < 8:
                    av_mms(6, g, 2 * g)
                    av_mms(6, g, 2 * g + 1)
                if g == 8:
                    # gpsimd queue: scalar-queue DMAs here would stall the
                    # exp stream (the scalar engine dispatches hwdge DMAs)
                    evac_norm(6, tail=False)
                if g >= 1:
                    av_mms(7, g, g - 1)
                if g >= 9:
                    pc, pco = s7_proj[g - 9]
                    proj_unit(pc, pco, ("av0", "av1")[g % 2])
            elif s > 0:
                av_mms(s - 1, g, g)
            if g < len(flist):
                fn, args = flist[g]
                fn(*args, ("f0", "f1")[fslot % 2])
                fslot += 1
            st = sps.tile([P, 1024], F32, tag="st", name=f"st{s}{g}")
            for h in range(2):
                nc.tensor.matmul(
                    st[:, h * 512 : (h + 1) * 512],
                    KT[p][h * HD : (h + 1) * HD, g * P : (g + 1) * P],
                    QT[p][h * HD : (h + 1) * HD, c * 512 : (c + 1) * 512],
                    start=True, stop=True,
                )
            nc.scalar.activation(e_cur[:, g, :], st, EXP, scale=1.0 / np.sqrt(HD))
        if s == 7:
            av_mms(7, NKT, NKT - 1)
        if 0 < s < 7:
            evac_norm(s - 1)

    # ---- tail: last av norm, remaining projections ----
    evac_norm(7, tail=True)
    # rotate over all four freed psum slots so the tail units pipeline
    tail_slots = ("av0", "av1", "f0", "f1")
    for n, (c, co) in enumerate([(2, 3), (3, 0), (3, 1), (3, 2), (3, 3)]):
        proj_unit(c, co, tail_slots[n % 4])

    # warm-up keep-alive (prevents DCE of the warm-up train)
    wdr = dram.tile([1, 8], F32, tag="wdr", name="wdr")
    nc.sync.dma_start(out=wdr, in_=wsb)


_CACHE = {}


def _get_nc():
    if "nc" in _CACHE:
        return _CACHE["nc"]
    nc = bacc.Bacc("TRN2", target_bir_lowering=False, debug=False)
    xb = nc.dram_tensor("xb", (L, C), BF16, kind="ExternalInput").ap()
    wg = nc.dram_tensor("wg", (C, 3 * GC), BF16, kind="ExternalInput").ap()
    wp = nc.dram_tensor("wp", (GC, C), BF16, kind="ExternalInput").ap()
    zt = nc.dram_tensor("zt", (C, L), BF16, kind="ExternalOutput").ap()
    from contextlib import ExitStack

    with tile.TileContext(nc) as tc, ExitStack() as ctx:
        _build_body(ctx, tc, xb, wg, wp, zt)
    nc.compile()
    _CACHE["nc"] = nc
    return nc


def make_in_maps(x, w_qkv, w_proj):
    """Slice full inputs into the 8 per-core input maps (pre-cast to bf16)."""
    import ml_dtypes

    bf = ml_dtypes.bfloat16
    x = np.asarray(x, dtype=np.float32).astype(bf)
    w_qkv = np.asarray(w_qkv, dtype=np.float32).astype(bf)
    w_proj = np.asarray(w_proj, dtype=np.float32).astype(bf)
    in_maps = []
    for c in range(NCORES):
        b, g = divmod(c, 2)
        cols = slice(g * GC, (g + 1) * GC)
        wg_c = np.concatenate(
            [w_qkv[:, cols], w_qkv[:, C + g * GC : C + (g + 1) * GC],
             w_qkv[:, 2 * C + g * GC : 2 * C + (g + 1) * GC]],
            axis=1,
        )
        in_maps.append(
            {
                "xb": np.ascontiguousarray(x[b]),
                "wg": np.ascontiguousarray(wg_c),
                "wp": np.ascontiguousarray(w_proj[cols, :]),
            }
        )
    return in_maps


def gather_output(results, b_proj):
    out = np.empty((B, L, C), dtype=np.float32)
    for b in range(B):
        z = (results[2 * b]["zt"].astype(np.float32)
             + results[2 * b + 1]["zt"].astype(np.float32))  # [C, L]
        out[b] = z.T + b_proj[None, :]
    return out


def kernel(x, w_qkv, b_qkv, w_proj, b_proj, _trace=False):
    assert np.abs(np.asarray(b_qkv)).max() == 0.0, "kernel assumes b_qkv == 0"
    nc = _get_nc()
    in_maps = make_in_maps(x, w_qkv, w_proj)
    res = bass_utils.run_bass_kernel_spmd(
        nc, in_maps, core_ids=list(range(NCORES)), trace=_trace
    )
    out = gather_output(res.results, np.asarray(b_proj, dtype=np.float32))
    if _trace:
        return out, res
    return out
